# revision 1
# baseline (speedup 1.0000x reference)
"""Trainium2 Bass kernel for nn_DechunkingLayer (ragged_sequence).

Reference semantics (per batch row):
    idx = clip(exclusive_cumsum(b), 0, NC - 1)          # [T]
    up[t]  = z[idx[t]]                                  # gather rows
    out[t] = p[t] * up[t] + (1 - p[t]) * up[t-1]        # EMA blend
    out[0] = up[0]

Sharding: pure data parallel over batch B=8 across the 8 NeuronCores.

Per-core plan (v6 — dedup gather + one-hot PE expansion + DVE-shuffle roll):
  idx is nondecreasing with average step 1/2, so the rows a 128-timestep
  tile needs are ~64 CONSECUTIVE z rows [idx[128k] .. idx[128k+127]].
  Gather only that window (~8.2 MB/core instead of 16 MB): the indirect
  DMA fetches rows bases_k + j(q) and marks slots past the window OOB
  (bounds_check + oob_is_err=False skips those reads — HW-verified to
  save the HBM bandwidth). j(q) interleaves the window across both
  partition halves so it spreads over all 16 SDMA engines. First ring
  lap: tiles 1..3 use CLAMPED indices (all 128 rows fetched), tiles
  4..7's bufs get a DVE memset during the ramp — either way the ring
  never holds uninitialized SBUF, so no 0*NaN hazard in the one-hot
  matmul. Tile 0's window is plainly
  z[0:128] (idx[t] <= t), loaded with a dependency-free direct DMA at
  t=0, and its W comes from a direct PE transpose+broadcast — so the
  pipeline starts as early as the scan allows.

  up = W1 @ zg on the PE (one-hot fp32 weights, bitwise exact;
  W1[q, i] = (j(q) == rank[i]), rank = idx - bases). fp32 streams at
  ~2 cyc/column: ~2.2 us/tile, the only PE work per tile.

  rolled[i] = up[i-1] comes from ONE DVE stream_shuffle (partition
  shift within 32-partition quadrants) plus a tiny stride-32 SBUF->SBUF
  DMA overwriting rows {0,32,64,96} from a precomputed boundary gather
  (BR[32a + k] = z[idxp[128k + 32a]], one [128, D] indirect gather for
  the whole kernel; idxp = min(s - b_shifted, NC-1) handles roll +
  clamp). Cross-tile rows are exact, so no epilogue pass.

  The per-tile rank rows (broadcast across partitions for the one-hot
  compare) are produced without per-tile PE work: one PE transpose ->
  DRAM scratch -> "replicate" indirect gathers whose indices are all
  equal (Tile tracks the DRAM W->R dependency across queues).

  Blends run one tile behind their shuffle (so the boundary-fix DMA
  latency never stalls the DVE queue) and alternate DVE / GpSimd to
  keep both vector engines under the ~2.3 us/tile pipeline pace.
  (gp_blend is off by default: gpsimd scalar_tensor_tensor fails the
  neuronxcc compile; DUP_TILES is empty: dup-gather `up` costs more on
  the DMA-bound side than it saves on the PE.)

  Blend math itself is elementwise fp32 in the same op order as jax,
  so the result is bitwise exact.
"""

import numpy as np

import concourse.bacc as bacc
import concourse.bass as bass
import concourse.mybir as mybir
import concourse.tile as tile
from concourse.bass import IndirectOffsetOnAxis
from concourse.bass_utils import run_bass_kernel_spmd
from concourse.masks import make_identity, make_upper_triangular

# Problem shape (hardcoded per harness contract).
B = 8          # batch rows == number of cores
T = 4096       # timesteps per row
NCH = 2048     # number of chunks (z rows)
D = 1024       # d_model
P = 128        # SBUF partitions
NT = T // P    # 32 tiles per core
NCOL = T // P  # 32 columns in the W layout
DH = D // 2    # matmul free-dim max for fp32 is 512

F32 = mybir.dt.float32
I32 = mybir.dt.int32

BIG = float(1 << 20)   # OOB marker offset for skipped gather rows
ZG_BUFS = 8            # gather window ring (how far gathers run ahead)
W_LOOK = 4             # W matrices built this many tiles ahead
NCHUNK = 4             # rank-table replication chunks (8 tiles each)
WARMUP_MM = 3          # PE warm-up matmuls against the HAM clock throttle
DUP_TILES = ()         # tiles whose `up` uses the duplicated-row gather
# stream_shuffle: out[32a+r] = in[32a+mask[r]]  ->  shift down by one
SHIFT_MASK = [0] + list(range(0, 31))


def build_bass(tile0_direct=True, gp_blend=False) -> bass.Bass:
    nc = bacc.Bacc()

    z = nc.dram_tensor("z", [NCH, D], F32, kind="ExternalInput")
    p = nc.dram_tensor("p", [T], F32, kind="ExternalInput")
    b = nc.dram_tensor("b", [T], I32, kind="ExternalInput")
    out = nc.dram_tensor("out", [T, D], F32, kind="ExternalOutput")
    # DRAM scratch for the rank-table broadcast roundtrip
    scratch = nc.dram_tensor("scratch", [NCOL, P], F32, kind="Internal")

    with tile.TileContext(nc) as tc:
        with (
            tc.tile_pool(name="setup", bufs=1) as sp,
            tc.tile_pool(name="psmall", bufs=2, space="PSUM") as pps,
            tc.tile_pool(name="pmm", bufs=3, space="PSUM") as pmm,
            tc.tile_pool(name="wpool", bufs=6) as wp,
            tc.tile_pool(name="main", bufs=4) as mp,
            tc.tile_pool(name="zg", bufs=ZG_BUFS) as zp,
        ):
            # ---- input loads + tile-0 window prefetch (no dependencies) ----
            b2d = b[:].rearrange("(j c) -> j c", c=P)          # [32, 128] DRAM
            p2d = p[:].rearrange("(j c) -> j c", c=P)

            b_nat_i = sp.tile([NCOL, P], I32)
            nc.sync.dma_start(out=b_nat_i[:], in_=b2d)
            zg0 = None
            if tile0_direct:
                zg0 = zp.tile([P, D], F32, tag="zg")  # tile-0 window z[0:128]
                nc.sync.dma_start(out=zg0[:], in_=z[0:P, :])
            p_nat = sp.tile([NCOL, P], F32)
            nc.sync.dma_start(out=p_nat[:], in_=p2d)

            # b_shifted[t] = b[t-1] (0 at t=0)
            bp_nat_i = sp.tile([NCOL, P], I32)
            nc.vector.memset(bp_nat_i[0:1, 0:1], 0)
            nc.sync.dma_start(out=bp_nat_i[:, 1:P], in_=b2d[:, 0 : P - 1])
            nc.sync.dma_start(
                out=bp_nat_i[1:NCOL, 0:1], in_=b2d[0 : NCOL - 1, P - 1 : P]
            )

            # ---- gpsimd constants (before the gather stream on its FIFO) ---
            tri_g = sp.tile([P, P], F32)     # tri[k, i] = 1 iff i > k
            make_upper_triangular(nc, tri_g[:], val=1.0, diag=False)
            ident_g = sp.tile([NCOL, NCOL], F32)
            make_identity(nc, ident_g[:])
            ident128_g = sp.tile([P, P], F32)
            make_identity(nc, ident128_g[:])
            tri32_g = sp.tile([NCOL, NCOL], F32)
            make_upper_triangular(nc, tri32_g[:], val=1.0, diag=False)
            iotap_i = sp.tile([P, 1], I32)   # iotap[q] = q
            nc.gpsimd.iota(iotap_i[:], pattern=[[0, 1]], base=0,
                           channel_multiplier=1)
            cidx_i = sp.tile([P, NCHUNK], I32)   # cidx[q, c] = c
            nc.gpsimd.iota(cidx_i[:], pattern=[[1, NCHUNK]], base=0,
                           channel_multiplier=0)
            jrow_i = sp.tile([P, NCOL], I32)     # jrow[q, j] = j
            nc.gpsimd.iota(jrow_i[:], pattern=[[1, NCOL]], base=0,
                           channel_multiplier=0)

            # ---- DVE: scan-critical copies first ---------------------------
            ident = sp.tile([NCOL, NCOL], F32)
            nc.vector.tensor_copy(out=ident[:], in_=ident_g[:])
            b_nat = sp.tile([NCOL, P], F32)
            nc.vector.tensor_copy(out=b_nat[:], in_=b_nat_i[:])
            bp_nat = sp.tile([NCOL, P], F32)
            nc.vector.tensor_copy(out=bp_nat[:], in_=bp_nat_i[:])
            tri = sp.tile([P, P], F32)
            nc.vector.tensor_copy(out=tri[:], in_=tri_g[:])
            tri32 = sp.tile([NCOL, NCOL], F32)
            nc.vector.tensor_copy(out=tri32[:], in_=tri32_g[:])
            ident128 = sp.tile([P, P], F32)
            nc.vector.tensor_copy(out=ident128[:], in_=ident128_g[:])
            iotap_f = sp.tile([P, 1], F32)
            nc.vector.tensor_copy(out=iotap_f[:], in_=iotap_i[:])

            ones_row = sp.tile([1, P], F32)
            nc.vector.memset(ones_row[:], 1.0)
            ones_col = sp.tile([P, 1], F32)
            nc.vector.memset(ones_col[:], 1.0)
            ones_pp = sp.tile([P, P], F32)
            nc.vector.memset(ones_pp[:], 1.0)
            warm_src = sp.tile([P, DH], F32)
            nc.vector.memset(warm_src[:], 1.0)

            # interleaved window offsets: j(q) = 2q (q<64), 2q-127 (q>=64)
            q2 = sp.tile([P, 1], F32)
            nc.vector.tensor_scalar_mul(out=q2[:], in0=iotap_f[:], scalar1=2.0)
            qm = sp.tile([P, 1], F32)
            nc.vector.tensor_scalar_min(out=qm[:], in0=iotap_f[:], scalar1=64.0)
            qhi = sp.tile([P, 1], F32)     # 1.0 iff q >= 64
            nc.vector.tensor_single_scalar(out=qhi[:], in_=qm[:], scalar=64.0,
                                           op=mybir.AluOpType.is_equal)
            iota2_col = sp.tile([P, 1], F32)   # j(q)
            nc.vector.scalar_tensor_tensor(
                out=iota2_col[:], in0=qhi[:], scalar=-127.0, in1=q2[:],
                op0=mybir.AluOpType.mult, op1=mybir.AluOpType.add,
            )
            # broadcast tiles (ACT): iota2_ff[q, i] = j(q); iotap_ff[q, i] = q
            iota2_ff = sp.tile([P, P], F32)
            nc.scalar.mul(out=iota2_ff[:], in_=ones_pp[:], mul=iota2_col[:])
            iotap_ff = sp.tile([P, P], F32)
            nc.scalar.mul(out=iotap_ff[:], in_=ones_pp[:], mul=iotap_f[:])

            # ---- PE transposes to W layout [128, 32]: t = 128*col + part ---
            bw_ps = pps.tile([P, NCOL], F32, space="PSUM", tag="small_ps")
            nc.tensor.transpose(out=bw_ps[:], in_=b_nat[:], identity=ident[:])
            b_w = sp.tile([P, NCOL], F32)
            nc.vector.tensor_copy(out=b_w[:], in_=bw_ps[:])

            bpw_ps = pps.tile([P, NCOL], F32, space="PSUM", tag="small_ps")
            nc.tensor.transpose(out=bpw_ps[:], in_=bp_nat[:], identity=ident[:])
            bp_w = sp.tile([P, NCOL], F32)
            nc.vector.tensor_copy(out=bp_w[:], in_=bpw_ps[:])

            pw_ps = pps.tile([P, NCOL], F32, space="PSUM", tag="small_ps")
            nc.tensor.transpose(out=pw_ps[:], in_=p_nat[:], identity=ident[:])
            p_w = sp.tile([P, NCOL], F32)
            nc.vector.tensor_copy(out=p_w[:], in_=pw_ps[:])
            # out[0] = up[0] exactly: p[0] = 1 so the blend is 1*up + 0*rolled
            nc.vector.memset(p_w[0:1, 0:1], 1.0)
            q_w = sp.tile([P, NCOL], F32)  # q = 1 - p
            nc.scalar.activation(
                out=q_w[:], in_=p_w[:],
                func=mybir.ActivationFunctionType.Copy, bias=1.0, scale=-1.0,
            )

            # ---- column offsets + full exclusive cumsum s ------------------
            totc_ps = pps.tile([NCOL, 1], F32, space="PSUM", tag="small_ps")
            nc.tensor.matmul(out=totc_ps[:], lhsT=b_w[:], rhs=ones_col[:],
                             start=True, stop=True)
            tot_col = sp.tile([NCOL, 1], F32)
            nc.vector.tensor_copy(out=tot_col[:], in_=totc_ps[:])
            cofs_ps = pps.tile([1, NCOL], F32, space="PSUM", tag="small_ps")
            nc.tensor.matmul(out=cofs_ps[:], lhsT=tot_col[:], rhs=tri32[:],
                             start=True, stop=True)
            colofs = sp.tile([1, NCOL], F32)
            nc.vector.tensor_copy(out=colofs[:], in_=cofs_ps[:])

            s_ps = pps.tile([P, NCOL], F32, space="PSUM", tag="small_ps")
            nc.tensor.matmul(out=s_ps[:], lhsT=tri[:], rhs=b_w[:],
                             start=True, stop=False)
            nc.tensor.matmul(out=s_ps[:], lhsT=ones_row[:], rhs=colofs[:],
                             start=False, stop=True)

            # idx = min(s, NCH-1); idxp = min(s - b_shifted, NCH-1)
            idx_f = sp.tile([P, NCOL], F32)
            nc.vector.tensor_scalar_min(out=idx_f[:], in0=s_ps[:],
                                        scalar1=float(NCH - 1))
            sprev_f = sp.tile([P, NCOL], F32)
            nc.vector.tensor_sub(out=sprev_f[:], in0=s_ps[:], in1=bp_w[:])
            idxp_f = sp.tile([P, NCOL], F32)
            nc.vector.tensor_scalar_min(out=idxp_f[:], in0=sprev_f[:],
                                        scalar1=float(NCH - 1))

            # ---- per-tile window bases / extents (gidx critical path) ------
            bases_row = sp.tile([1, NCOL], F32)   # idx[128k]
            nc.vector.tensor_copy(out=bases_row[:], in_=idx_f[0:1, :])
            last_row = sp.tile([1, NCOL], F32)    # idx[128k+127] via tiny DMA
            nc.scalar.dma_start(out=last_row[:], in_=idx_f[P - 1 : P, :])

            basesb_ps = pps.tile([P, NCOL], F32, space="PSUM", tag="small_ps")
            nc.tensor.matmul(out=basesb_ps[:], lhsT=ones_row[:],
                             rhs=bases_row[:], start=True, stop=True)
            lastb_ps = pps.tile([P, NCOL], F32, space="PSUM", tag="small_ps")
            nc.tensor.matmul(out=lastb_ps[:], lhsT=ones_row[:],
                             rhs=last_row[:], start=True, stop=True)

            # rank table [128, 32] (one-hot positions within the window)
            rank1 = sp.tile([P, NCOL], F32)
            nc.vector.tensor_sub(out=rank1[:], in0=idx_f[:], in1=basesb_ps[:])

            # gather indices: gidx[q, k] = bases[k] + j(q); marked OOB past
            # last (steady laps) or clamped to NCH-1 (first lap, NaN safety)
            graw = sp.tile([P, NCOL], F32)
            nc.vector.tensor_scalar_add(out=graw[:], in0=basesb_ps[:],
                                        scalar1=iota2_col[:])
            delta = sp.tile([P, NCOL], F32)
            nc.vector.tensor_sub(out=delta[:], in0=lastb_ps[:], in1=graw[:])
            dmin = sp.tile([P, NCOL], F32)
            nc.vector.tensor_scalar_min(out=dmin[:], in0=delta[:], scalar1=0.0)
            vld = sp.tile([P, NCOL], F32)
            nc.vector.tensor_single_scalar(out=vld[:], in_=dmin[:], scalar=0.0,
                                           op=mybir.AluOpType.is_equal)
            graw_big = sp.tile([P, NCOL], F32)
            nc.vector.tensor_scalar_add(out=graw_big[:], in0=graw[:],
                                        scalar1=BIG)
            gidx_f = sp.tile([P, NCOL], F32)
            nc.vector.scalar_tensor_tensor(
                out=gidx_f[:], in0=vld[:], scalar=-BIG, in1=graw_big[:],
                op0=mybir.AluOpType.mult, op1=mybir.AluOpType.add,
            )
            gidx_i = sp.tile([P, NCOL], I32)
            nc.vector.tensor_copy(out=gidx_i[:], in_=gidx_f[:])
            gidxc_f = sp.tile([P, NCOL], F32)   # clamped (first ring lap)
            nc.vector.tensor_scalar_min(out=gidxc_f[:], in0=graw[:],
                                        scalar1=float(NCH - 1))
            gidxc_i = sp.tile([P, NCOL], I32)
            nc.vector.tensor_copy(out=gidxc_i[:], in_=gidxc_f[:])

            # rank table -> DRAM for the replicate-gather
            r1t_ps = pps.tile([NCOL, P], F32, space="PSUM", tag="small_ps")
            nc.tensor.transpose(out=r1t_ps[:], in_=rank1[:],
                                identity=ident128[:])
            rank1t = sp.tile([NCOL, P], F32)
            nc.vector.tensor_copy(out=rank1t[:], in_=r1t_ps[:])

            # tile-0 W on the direct path (its window is the plain z[0:128]):
            # rank_0 = idx[:, 0]; transpose to a row, broadcast, compare.
            w1_0 = None
            if tile0_direct:
                r0t_ps = pps.tile([1, P], F32, space="PSUM", tag="small_ps")
                nc.tensor.transpose(out=r0t_ps[:], in_=idx_f[:, 0:1],
                                    identity=ident128[:])
                r0row = sp.tile([1, P], F32)
                nc.vector.tensor_copy(out=r0row[:], in_=r0t_ps[:])
                rb0_ps = pps.tile([P, P], F32, space="PSUM", tag="small_ps")
                nc.tensor.matmul(out=rb0_ps[:], lhsT=ones_row[:], rhs=r0row[:],
                                 start=True, stop=True)
                w1_0 = wp.tile([P, P], F32, tag="w1")
                nc.vector.tensor_tensor(out=w1_0[:], in0=iotap_ff[:],
                                        in1=rb0_ps[:],
                                        op=mybir.AluOpType.is_equal)

            # boundary-roll gather index: BR[32a + k] = z[idxp[128k + 32a]]
            # (diagonal extraction: quadrant-broadcast idxp rows {0,32,64,96},
            #  then pick free element q mod 32 per partition)
            jrow_f = sp.tile([P, NCOL], F32)
            nc.vector.tensor_copy(out=jrow_f[:], in_=jrow_i[:])
            qa = sp.tile([P, 1], F32)      # floor(q / 32) via 3 thresholds
            nc.vector.memset(qa[:], 0.0)
            for thr in (32.0, 64.0, 96.0):
                qt = sp.tile([P, 1], F32, name=f"qt{int(thr)}")
                nc.vector.tensor_scalar_min(out=qt[:], in0=iotap_f[:],
                                            scalar1=thr)
                qi = sp.tile([P, 1], F32, name=f"qi{int(thr)}")
                nc.vector.tensor_single_scalar(out=qi[:], in_=qt[:], scalar=thr,
                                               op=mybir.AluOpType.is_equal)
                nc.vector.tensor_add(out=qa[:], in0=qa[:], in1=qi[:])
            qmod = sp.tile([P, 1], F32)
            nc.vector.scalar_tensor_tensor(
                out=qmod[:], in0=qa[:], scalar=-32.0, in1=iotap_f[:],
                op0=mybir.AluOpType.mult, op1=mybir.AluOpType.add,
            )
            eye = sp.tile([P, NCOL], F32)
            nc.vector.tensor_scalar(out=eye[:], in0=jrow_f[:], scalar1=qmod[:],
                                    scalar2=None,
                                    op0=mybir.AluOpType.is_equal)
            bq = sp.tile([P, NCOL], F32)
            nc.vector.stream_shuffle(out=bq[:], in_=idxp_f[:], mask=[0] * 32)
            bprod = sp.tile([P, NCOL], F32)
            nc.vector.tensor_mul(out=bprod[:], in0=bq[:], in1=eye[:])
            bidx_f = sp.tile([P, 1], F32)
            nc.vector.tensor_reduce(out=bidx_f[:], in_=bprod[:],
                                    axis=mybir.AxisListType.X,
                                    op=mybir.AluOpType.add)
            bidx_i = sp.tile([P, 1], I32)
            nc.vector.tensor_copy(out=bidx_i[:], in_=bidx_f[:])

            idx_i = sp.tile([P, NCOL], I32)   # for DUP_TILES' direct gather
            nc.vector.tensor_copy(out=idx_i[:], in_=idx_f[:])

            # gather ring bufs for the first lap: tiles 1-3 use clamped
            # (fully-written) windows, tiles 4-7 get a DVE memset during the
            # ramp so their OOB-marked gathers never expose stale SBUF.
            zg_ring = {}
            for i in range(1, ZG_BUFS):
                t0r = zp.tile([P, D], F32, tag="zg", name=f"zgr{i}")
                if i >= 4:
                    nc.vector.memset(t0r[:], 0.0)
                zg_ring[i] = t0r

            # ---- gpsimd stream: scratch store, chunks, BR, early windows ---
            nc.gpsimd.dma_start(out=scratch[:], in_=rank1t[:])
            scr_flat = scratch[:].rearrange("(a k) c -> a (k c)", a=NCHUNK)

            rbs = [sp.tile([P, (NCOL * P) // NCHUNK], F32, name=f"rb{c}")
                   for c in range(NCHUNK)]

            def chunk_gather(c):
                nc.gpsimd.indirect_dma_start(
                    out=rbs[c][:], out_offset=None, in_=scr_flat,
                    in_offset=IndirectOffsetOnAxis(ap=cidx_i[:, c : c + 1],
                                                   axis=0),
                )

            zgs = {}
            if tile0_direct:
                zgs[0] = zg0

            def window_gather(k):
                if k >= NT:
                    return
                zg = (zg_ring[k] if 0 < k < ZG_BUFS
                      else zp.tile([P, D], F32, tag="zg", name=f"zgw{k}"))
                if 0 < k < 4:
                    # first ring lap head: clamped, fully written (NaN safety)
                    nc.gpsimd.indirect_dma_start(
                        out=zg[:], out_offset=None, in_=z[:],
                        in_offset=IndirectOffsetOnAxis(ap=gidxc_i[:, k : k + 1],
                                                       axis=0),
                    )
                else:
                    nc.gpsimd.indirect_dma_start(
                        out=zg[:], out_offset=None, in_=z[:],
                        in_offset=IndirectOffsetOnAxis(ap=gidx_i[:, k : k + 1],
                                                       axis=0),
                        bounds_check=NCH - 1, oob_is_err=False,
                    )
                zgs[k] = zg

            chunk_gather(0)
            chunk_gather(1)
            br = sp.tile([P, D], F32)
            nc.gpsimd.indirect_dma_start(
                out=br[:], out_offset=None, in_=z[:],
                in_offset=IndirectOffsetOnAxis(ap=bidx_i[:, 0:1], axis=0),
            )
            for k in range(1 if tile0_direct else 0, ZG_BUFS):
                window_gather(k)

            # ---- W build: one DVE is_equal per tile ------------------------
            def build_w(k):
                if k >= NT or k in DUP_TILES:
                    return None
                o = (k % (NT // NCHUNK)) * P
                w1 = wp.tile([P, P], F32, tag="w1", name=f"w1_{k}")
                nc.vector.tensor_tensor(out=w1[:], in0=iota2_ff[:],
                                        in1=rbs[k // (NT // NCHUNK)][:, o : o + P],
                                        op=mybir.AluOpType.is_equal)
                return w1

            ws = {}
            if tile0_direct:
                ws[0] = w1_0
            for k in range(0 if not tile0_direct else 1, W_LOOK):
                ws[k] = build_w(k)

            # PE warm-up: a dense burst against the HAM clock gate right
            # before the main matmuls start.
            for w in range(WARMUP_MM):
                wps = pmm.tile([P, D], F32, space="PSUM", tag="mm")
                nc.tensor.matmul(out=wps[:, 0:DH], lhsT=tri[:], rhs=warm_src[:],
                                 start=True, stop=True, skip_group_check=True)
                if w == WARMUP_MM - 1:
                    warm_sink = sp.tile([1, 1], F32)
                    nc.vector.tensor_copy(out=warm_sink[:], in_=wps[0:1, 0:1])

            # ---- main loop: expand, shuffle-roll, blend (1 behind), store --
            pend = {}   # k -> (roll, t1)

            def emit_blend(j):
                roll_j, t1_j = pend.pop(j)
                o = mp.tile([P, D], F32, tag="o", name=f"o{j}")
                eng = nc.gpsimd if (gp_blend and j % 2 == 1) else nc.vector
                eng.scalar_tensor_tensor(
                    out=o[:], in0=roll_j[:], scalar=q_w[:, j : j + 1],
                    in1=t1_j[:],
                    op0=mybir.AluOpType.mult, op1=mybir.AluOpType.add,
                )
                nc.sync.dma_start(out=out[j * P : (j + 1) * P, :], in_=o[:])

            for k in range(NT):
                window_gather(k + ZG_BUFS)       # keep the ring k+8 ahead
                if k == 2:
                    chunk_gather(2)
                if k == 10:
                    chunk_gather(3)
                if k + W_LOOK < NT:
                    ws[k + W_LOOK] = build_w(k + W_LOOK)
                w1 = ws.pop(k)
                zg = zgs.pop(k)

                if k in DUP_TILES:
                    up_src = zg          # gathered with duplicates: up in SBUF
                else:
                    up_ps = pmm.tile([P, D], F32, space="PSUM", tag="mm",
                                     name=f"up{k}")
                    for h in range(2):
                        sl = slice(h * DH, (h + 1) * DH)
                        nc.tensor.matmul(out=up_ps[:, sl], lhsT=w1[:],
                                         rhs=zg[:, sl], start=True, stop=True,
                                         skip_group_check=True)
                    up_src = up_ps

                if k >= 1:
                    emit_blend(k - 1)    # one tile behind: its fix-DMA is done

                roll = mp.tile([P, D], F32, tag="roll", name=f"roll{k}")
                nc.vector.stream_shuffle(out=roll[:], in_=up_src[:],
                                         mask=SHIFT_MASK)
                # fix quadrant-boundary rows {0,32,64,96} from BR
                nc.scalar.dma_start(out=roll[0:P:NCOL, :],
                                    in_=br[k : P : NCOL, :])

                t1 = mp.tile([P, D], F32, tag="t1", name=f"t1_{k}")
                nc.scalar.mul(out=t1[:], in_=up_src[:], mul=p_w[:, k : k + 1])
                pend[k] = (roll, t1)

            emit_blend(NT - 1)

    nc.finalize()
    return nc


_NC_CACHE = None


def _get_nc() -> bass.Bass:
    global _NC_CACHE
    if _NC_CACHE is None:
        _NC_CACHE = build_bass()
    return _NC_CACHE


def make_in_maps(z: np.ndarray, p: np.ndarray, b: np.ndarray) -> list[dict]:
    return [
        {
            "z": np.ascontiguousarray(z[i], dtype=np.float32),
            "p": np.ascontiguousarray(p[i], dtype=np.float32),
            "b": np.ascontiguousarray(b[i], dtype=np.int32),
        }
        for i in range(B)
    ]


def kernel(z, p, b, original_len=None, **_unused) -> np.ndarray:
    z = np.asarray(z, dtype=np.float32)
    p = np.asarray(p, dtype=np.float32)
    b = np.asarray(b, dtype=np.int32)
    assert z.shape == (B, NCH, D) and p.shape == (B, T) and b.shape == (B, T)

    nc = _get_nc()
    res = run_bass_kernel_spmd(nc, make_in_maps(z, p, b), list(range(B)))
    return np.stack([r["out"] for r in res.results], axis=0)

